# revision 4
# baseline (speedup 1.0000x reference)
"""FlowNetC correlation kernel for Trainium2 (8 NeuronCores, SPMD).

Problem: input1/input2 [B=8, C=256, H=48, W=64] fp32.
out[b, d, y, x] = (1/C) * sum_c in1[b,c,y,x] * in2[b,c,y+dy,x+dx]
with d = dyi*21 + dxi, dy = 2*dyi - 20, dx = 2*dxi - 20 (zero outside bounds).

Strategy:
  - Data-parallel over batch: one sample per NeuronCore.
  - Per-pixel dot products over C map to Gram-matrix *bands* on the PE:
    block M = 128 stationary columns = (4 same-parity y) x (32 same-parity x),
    moving columns = (valid same-parity y' rows) x (32 same-parity x').
    Displacements have stride 2 so parities never mix.
  - The device writes the Gram band blocks to DRAM in their natural matmul
    layout (pure large contiguous DMAs); the host gathers the 441 diagonals
    into the [B, 441, H, W] output with one precomputed numpy index table.
"""

import os
import numpy as np

H, W, C = 48, 64, 256
GRID = 21  # displacement grid per axis
NYH = H // 2  # 24 same-parity y values
NXH = W // 2  # 32 same-parity x values
NG = 6  # y-groups of 4 same-parity rows each

# per y-group g (4 same-parity rows 4g..4g+3 in parity space), the valid
# B-row window in parity space: j in [J0[g], J1[g]]
J0 = [max(0, 4 * g - 10) for g in range(NG)]
J1 = [min(NYH - 1, 4 * g + 13) for g in range(NG)]
ROWS = [j1 - j0 + 1 for j0, j1 in zip(J0, J1)]  # [14, 18, 22, 22, 18, 14]
CUM = np.concatenate([[0], np.cumsum(ROWS)])  # [0,14,32,54,76,94,108]
COLS_PER_Q = int(CUM[-1]) * NXH  # 108*32 = 3456 columns per (yp,xp) pair
N_COLS = 4 * COLS_PER_Q  # 13824
MM_DTYPE = os.environ.get("KERNEL_MM_DTYPE", "float32r")


def _chunks(nrows):
    """Split a row count into PSUM-bank-sized chunks (<=16 rows = 512 cols)."""
    if nrows <= 16:
        return [nrows]
    n = (nrows + 15) // 16
    base = nrows // n
    rem = nrows - base * n
    return [base + (1 if i < rem else 0) for i in range(n)]


_nc_cache = {}


def _build_nc():
    key = "nc"
    if key in _nc_cache:
        return _nc_cache[key]
    import concourse.bacc as bacc
    import concourse.bass as bass
    import concourse.mybir as mybir
    import concourse.tile as tile

    nc = bacc.Bacc("TRN2", target_bir_lowering=False, debug=False)
    in1 = nc.dram_tensor("input1", [C, H * W], mybir.dt.float32, kind="ExternalInput")
    in2 = nc.dram_tensor("input2", [C, H * W], mybir.dt.float32, kind="ExternalInput")
    staged = nc.dram_tensor(
        "staged", [128, N_COLS], mybir.dt.float32, kind="ExternalOutput"
    )

    mm_dt = getattr(mybir.dt, MM_DTYPE)

    with tile.TileContext(nc) as tc:
        with (
            tc.tile_pool(name="inp", bufs=1) as inp_pool,
            tc.tile_pool(name="psum", bufs=8, space="PSUM") as psum_pool,
            tc.tile_pool(name="stage", bufs=8) as stage_pool,
        ):
            a_sb = inp_pool.tile([128, 2 * H * W], mybir.dt.float32, tag="a")
            b_sb = inp_pool.tile([128, 2 * H * W], mybir.dt.float32, tag="b")
            # split each input load in two so compute can start after half
            for k in range(2):
                nc.sync.dma_start(
                    out=a_sb[:, k * H * W : (k + 1) * H * W],
                    in_=in1[k * 128 : (k + 1) * 128, :],
                )
                nc.sync.dma_start(
                    out=b_sb[:, k * H * W : (k + 1) * H * W],
                    in_=in2[k * 128 : (k + 1) * 128, :],
                )

            # host pre-shuffles inputs to parity-major free layout:
            # free dim = k*3072 + yp*1536 + xp*768 + yh*32 + xh
            # so every matmul operand below is one contiguous run.
            a_v = a_sb[:].rearrange(
                "c (k yp xp yh xh) -> c k yp xp yh xh", k=2, yh=NYH, yp=2, xh=NXH, xp=2
            )
            b_v = b_sb[:].rearrange(
                "c (k yp xp yh xh) -> c k yp xp yh xh", k=2, yh=NYH, yp=2, xh=NXH, xp=2
            )
            if mm_dt != mybir.dt.float32:
                a_v = a_v.bitcast(mm_dt)
                b_v = b_v.bitcast(mm_dt)

            col0 = 0
            for yp in range(2):
                for xp in range(2):
                    for g in range(NG):
                        chunk_rows = _chunks(ROWS[g])
                        # per-k weight reuse: all chunks of one k before next k
                        psum_tiles = []
                        for nr in chunk_rows:
                            pt = psum_pool.tile(
                                [128, nr * NXH], mybir.dt.float32, tag="pt"
                            )
                            psum_tiles.append(pt)
                        for k in range(2):
                            lhsT = a_v[:, k, yp, xp, 4 * g : 4 * g + 4, :]
                            ja = J0[g]
                            for ci, nr in enumerate(chunk_rows):
                                rhs = b_v[:, k, yp, xp, ja : ja + nr, :]
                                nc.tensor.matmul(
                                    psum_tiles[ci][:],
                                    lhsT,
                                    rhs,
                                    start=(k == 0),
                                    stop=(k == 1),
                                )
                                ja += nr
                        for ci, nr in enumerate(chunk_rows):
                            n = nr * NXH
                            st = stage_pool.tile([128, n], mybir.dt.float32, tag="st")
                            nc.vector.tensor_scalar_mul(
                                st[:], psum_tiles[ci][:], 1.0 / C
                            )
                            nc.sync.dma_start(
                                out=staged[:, col0 : col0 + n], in_=st[:]
                            )
                            col0 += n
            assert col0 == N_COLS, col0

    nc.compile()
    _nc_cache[key] = nc
    return nc


_idx_cache = {}


def _host_index():
    """Precompute gather index + validity mask mapping staged -> output."""
    if "idx" in _idx_cache:
        return _idx_cache["idx"]
    d = np.arange(441)
    dy = 2 * (d // GRID) - 20
    dx = 2 * (d % GRID) - 20
    y = np.arange(H)
    x = np.arange(W)
    DY = dy[:, None, None]
    DX = dx[:, None, None]
    Y = y[None, :, None]
    X = x[None, None, :]
    Yp = Y + DY
    Xp = X + DX
    valid = (Yp >= 0) & (Yp < H) & (Xp >= 0) & (Xp < W)
    Ypc = np.clip(Yp, 0, H - 1)
    Xpc = np.clip(Xp, 0, W - 1)
    yp = Y % 2
    xp = X % 2
    q = yp * 2 + xp
    g = (Y // 2) // 4
    i = (Y // 2) % 4
    xe = X // 2
    j = Ypc // 2
    j0 = np.asarray(J0)[g]
    jj = j - j0
    xpe = Xpc // 2
    cum = np.asarray(CUM[:-1])[g]
    col = q * COLS_PER_Q + (cum + jj) * NXH + xpe
    m = i * NXH + xe
    lin = m * N_COLS + col
    lin = np.where(valid, lin, 0).astype(np.int64)
    out = (lin, valid.astype(np.float32))
    _idx_cache["idx"] = out
    return out


def kernel(input1: np.ndarray, input2: np.ndarray) -> np.ndarray:
    import sys

    for p in ("/opt/trn_rl_repo", "/root/.axon_site/_ro/trn_rl_repo"):
        if os.path.isdir(p) and p not in sys.path:
            sys.path.append(p)
    from concourse import bass_utils

    B = input1.shape[0]
    input1 = np.ascontiguousarray(input1, dtype=np.float32)
    input2 = np.ascontiguousarray(input2, dtype=np.float32)

    def _shuffle(x):
        # [C,H,W] -> parity-major [C, yp, xp, yh, xh] -> [C, H*W]
        v = x.reshape(C, NYH, 2, NXH, 2).transpose(0, 2, 4, 1, 3)
        return np.ascontiguousarray(v).reshape(C, H * W)

    nc = _build_nc()
    in_maps = [
        {
            "input1": _shuffle(input1[b]),
            "input2": _shuffle(input2[b]),
        }
        for b in range(B)
    ]
    trace = os.environ.get("KERNEL_TRACE", "0") == "1"
    res = bass_utils.run_bass_kernel_spmd(
        nc, in_maps, core_ids=list(range(B)), trace=trace
    )
    kernel.last_exec_time_ns = res.exec_time_ns
    kernel.last_profile = res.profile_json

    lin, valid = _host_index()
    out = np.empty((B, 441, H, W), dtype=np.float32)
    for b in range(B):
        flat = np.asarray(res.results[b]["staged"]).reshape(-1)
        out[b] = flat[lin] * valid
    return out


kernel.last_exec_time_ns = None
kernel.last_profile = None


# revision 7
# speedup vs baseline: 1.2461x; 1.2461x over previous
"""FlowNetC correlation kernel for Trainium2 (8 NeuronCores, SPMD).

Problem: input1/input2 [B=8, C=256, H=48, W=64] fp32.
out[b, d, y, x] = (1/C) * sum_c in1[b,c,y,x] * in2[b,c,y+dy,x+dx]
with d = dyi*21 + dxi, dy = 2*dyi - 20, dx = 2*dxi - 20 (zero outside bounds).

Strategy:
  - Data-parallel over batch: one sample per NeuronCore.
  - Per-pixel dot products over C map to Gram-matrix *bands* on the PE:
    block M = 128 stationary columns = (4 same-parity y) x (32 same-parity x),
    moving columns = (valid same-parity y' rows) x (32 same-parity x').
    Displacements have stride 2 so parities never mix.
  - The device writes the Gram band blocks to DRAM in their natural matmul
    layout (pure large contiguous DMAs); the host gathers the 441 diagonals
    into the [B, 441, H, W] output with one precomputed numpy index table.
"""

import os
import numpy as np

H, W, C = 48, 64, 256
GRID = 21  # displacement grid per axis
NYH = H // 2  # 24 same-parity y values
NXH = W // 2  # 32 same-parity x values
NG = 6  # y-groups of 4 same-parity rows each

# per y-group g (4 same-parity rows 4g..4g+3 in parity space), the valid
# B-row window in parity space: j in [J0[g], J1[g]]
J0 = [max(0, 4 * g - 10) for g in range(NG)]
J1 = [min(NYH - 1, 4 * g + 13) for g in range(NG)]
ROWS = [j1 - j0 + 1 for j0, j1 in zip(J0, J1)]  # [14, 18, 22, 22, 18, 14]
CUM = np.concatenate([[0], np.cumsum(ROWS)])  # [0,14,32,54,76,94,108]
COLS_PER_Q = int(CUM[-1]) * NXH  # 108*32 = 3456 columns per (yp,xp) pair
N_COLS = 4 * COLS_PER_Q  # 13824
MM_DTYPE = os.environ.get("KERNEL_MM_DTYPE", "float32r")


def _chunks(nrows):
    """Split a row count into PSUM-bank-sized chunks (<=16 rows = 512 cols)."""
    if nrows <= 16:
        return [nrows]
    n = (nrows + 15) // 16
    base = nrows // n
    rem = nrows - base * n
    return [base + (1 if i < rem else 0) for i in range(n)]


_nc_cache = {}


def _build_nc():
    key = "nc"
    if key in _nc_cache:
        return _nc_cache[key]
    import concourse.bacc as bacc
    import concourse.bass as bass
    import concourse.mybir as mybir
    import concourse.tile as tile

    nc = bacc.Bacc("TRN2", target_bir_lowering=False, debug=False)
    mm_dt = getattr(mybir.dt, MM_DTYPE)
    in1 = nc.dram_tensor("input1", [C, H * W], mm_dt, kind="ExternalInput")
    in2 = nc.dram_tensor("input2", [C, H * W], mm_dt, kind="ExternalInput")
    staged = nc.dram_tensor(
        "staged", [128, N_COLS], mybir.dt.float32, kind="ExternalOutput"
    )

    with tile.TileContext(nc) as tc:
        with (
            tc.tile_pool(name="inp", bufs=1) as inp_pool,
            tc.tile_pool(name="psum", bufs=8, space="PSUM") as psum_pool,
            tc.tile_pool(name="stage", bufs=8) as stage_pool,
        ):
            a_sb = inp_pool.tile([128, 2 * H * W], mm_dt, tag="a")
            b_sb = inp_pool.tile([128, 2 * H * W], mm_dt, tag="b")
            # split each input load in two so compute can start after half
            for k in range(2):
                nc.sync.dma_start(
                    out=a_sb[:, k * H * W : (k + 1) * H * W],
                    in_=in1[k * 128 : (k + 1) * 128, :],
                )
                nc.sync.dma_start(
                    out=b_sb[:, k * H * W : (k + 1) * H * W],
                    in_=in2[k * 128 : (k + 1) * 128, :],
                )

            # host pre-shuffles inputs to parity-major free layout:
            # free dim = k*3072 + yp*1536 + xp*768 + yh*32 + xh
            # so every matmul operand below is one contiguous run.
            a_v = a_sb[:].rearrange(
                "c (k yp xp yh xh) -> c k yp xp yh xh", k=2, yh=NYH, yp=2, xh=NXH, xp=2
            )
            b_v = b_sb[:].rearrange(
                "c (k yp xp yh xh) -> c k yp xp yh xh", k=2, yh=NYH, yp=2, xh=NXH, xp=2
            )

            col0 = 0
            for yp in range(2):
                for xp in range(2):
                    for g in range(NG):
                        chunk_rows = _chunks(ROWS[g])
                        # per-k weight reuse: all chunks of one k before next k
                        psum_tiles = []
                        for nr in chunk_rows:
                            pt = psum_pool.tile(
                                [128, nr * NXH], mybir.dt.float32, tag="pt"
                            )
                            psum_tiles.append(pt)
                        for k in range(2):
                            lhsT = a_v[:, k, yp, xp, 4 * g : 4 * g + 4, :]
                            ja = J0[g]
                            for ci, nr in enumerate(chunk_rows):
                                rhs = b_v[:, k, yp, xp, ja : ja + nr, :]
                                nc.tensor.matmul(
                                    psum_tiles[ci][:],
                                    lhsT,
                                    rhs,
                                    start=(k == 0),
                                    stop=(k == 1),
                                )
                                ja += nr
                        for ci, nr in enumerate(chunk_rows):
                            n = nr * NXH
                            st = stage_pool.tile([128, n], mybir.dt.float32, tag="st")
                            nc.vector.tensor_scalar_mul(
                                st[:], psum_tiles[ci][:], 1.0 / C
                            )
                            nc.sync.dma_start(
                                out=staged[:, col0 : col0 + n], in_=st[:]
                            )
                            col0 += n
            assert col0 == N_COLS, col0

    nc.compile()
    _nc_cache[key] = nc
    return nc


_idx_cache = {}


def _host_index():
    """Precompute gather index + validity mask mapping staged -> output."""
    if "idx" in _idx_cache:
        return _idx_cache["idx"]
    d = np.arange(441)
    dy = 2 * (d // GRID) - 20
    dx = 2 * (d % GRID) - 20
    y = np.arange(H)
    x = np.arange(W)
    DY = dy[:, None, None]
    DX = dx[:, None, None]
    Y = y[None, :, None]
    X = x[None, None, :]
    Yp = Y + DY
    Xp = X + DX
    valid = (Yp >= 0) & (Yp < H) & (Xp >= 0) & (Xp < W)
    Ypc = np.clip(Yp, 0, H - 1)
    Xpc = np.clip(Xp, 0, W - 1)
    yp = Y % 2
    xp = X % 2
    q = yp * 2 + xp
    g = (Y // 2) // 4
    i = (Y // 2) % 4
    xe = X // 2
    j = Ypc // 2
    j0 = np.asarray(J0)[g]
    jj = j - j0
    xpe = Xpc // 2
    cum = np.asarray(CUM[:-1])[g]
    col = q * COLS_PER_Q + (cum + jj) * NXH + xpe
    m = i * NXH + xe
    lin = m * N_COLS + col
    lin = np.where(valid, lin, 0).astype(np.int64)
    out = (lin, valid.astype(np.float32))
    _idx_cache["idx"] = out
    return out


def kernel(input1: np.ndarray, input2: np.ndarray) -> np.ndarray:
    import sys

    for p in ("/opt/trn_rl_repo", "/root/.axon_site/_ro/trn_rl_repo"):
        if os.path.isdir(p) and p not in sys.path:
            sys.path.append(p)
    from concourse import bass_utils

    B = input1.shape[0]
    input1 = np.ascontiguousarray(input1, dtype=np.float32)
    input2 = np.ascontiguousarray(input2, dtype=np.float32)

    def _shuffle(x):
        # [C,H,W] -> parity-major [C, yp, xp, yh, xh] -> [C, H*W]
        v = x.reshape(C, NYH, 2, NXH, 2).transpose(0, 2, 4, 1, 3)
        return np.ascontiguousarray(v).reshape(C, H * W)

    nc = _build_nc()
    in_maps = [
        {
            "input1": _shuffle(input1[b]),
            "input2": _shuffle(input2[b]),
        }
        for b in range(B)
    ]
    trace = os.environ.get("KERNEL_TRACE", "0") == "1"
    res = bass_utils.run_bass_kernel_spmd(
        nc, in_maps, core_ids=list(range(B)), trace=trace
    )
    kernel.last_exec_time_ns = res.exec_time_ns
    kernel.last_profile = res.profile_json

    lin, valid = _host_index()
    out = np.empty((B, 441, H, W), dtype=np.float32)
    for b in range(B):
        flat = np.asarray(res.results[b]["staged"]).reshape(-1)
        out[b] = flat[lin] * valid
    return out


kernel.last_exec_time_ns = None
kernel.last_profile = None
